# revision 24
# baseline (speedup 1.0000x reference)
"""DeepAR (2-layer LSTM, H=1024, B=128, T=160) Trainium2 kernel.

Strategy: tensor-parallel over hidden units across 8 cores (each core owns a
128-unit slice of each layer -> 512 gate rows), full batch B=128 on every
core so the PE array's 128 partitions stay full (batch-major gates,
activation-stationary / weight-streaming matmuls).  The time recurrence is
serial; per step each core computes its h0/h1 slices and two AllGathers
rebuild the full hidden vectors on every core.  The DeepAR mu feedback is
reduced to a rank-2 matmul via host-precomputed u = W_lab @ W_tgt^T and
v = W_lab @ b_tgt, so the only mu-dependent device work per step is one
K=2 matmul; everything else pre-accumulates into PSUM during the AllGather
windows.  mu/sigma projections ride the gathered h chunks as tiny N=2
matmuls; softplus is applied on the host (sigma is output-only).
"""

import sys

import ml_dtypes
import numpy as np

sys.path.insert(0, "/opt/trn_rl_repo")

import concourse.bass as bass
import concourse.mybir as mybir
import concourse.tile as tile
from concourse import bacc
from concourse.bass import ts
from concourse.bass_utils import run_bass_kernel_spmd
from concourse.masks import make_identity

F32 = mybir.dt.float32
BF16 = mybir.dt.bfloat16
AF = mybir.ActivationFunctionType

B = 128
T = 160
F = 512
E = 64
H = 1024
NCORE = 8
HS = H // NCORE          # hidden units per core per layer
G = 4 * HS               # gate rows per core (i|f|o|g blocks of 128)
KF = F // 128            # feat chunks
KH = H // 128            # hidden chunks
C_IH0, C_HH0, C_IH1, C_HH1 = 0, KF, KF + KH, KF + 2 * KH
NWCH = KF + 3 * KH       # 28 weight chunks of [128, G]


def gate_rows(j):
    """Gate rows (torch order i,f,g,o) owned by core j, reordered to i,f,o,g
    so that sigmoid gates occupy contiguous psum columns 0:384."""
    sl = np.arange(HS * j, HS * j + HS)
    return np.concatenate([0 * H + sl, 1 * H + sl, 3 * H + sl, 2 * H + sl])


def build(T_steps=T, mode="full"):
    nc = bacc.Bacc(num_devices=NCORE)

    wall = nc.dram_tensor("wall", [128, NWCH * G], BF16, kind="ExternalInput")
    wlab = nc.dram_tensor("wlab", [E, G], BF16, kind="ExternalInput")
    uv = nc.dram_tensor("uv", [2, G], BF16, kind="ExternalInput")
    bias01 = nc.dram_tensor("bias01", [1, 2 * G], BF16, kind="ExternalInput")
    pw0 = nc.dram_tensor("pw0", [128, 2 * KH], BF16, kind="ExternalInput")
    pw1 = nc.dram_tensor("pw1", [128, 2 * KH], BF16, kind="ExternalInput")
    msbias = nc.dram_tensor("msbias", [2, 1], F32, kind="ExternalInput")
    featT = nc.dram_tensor("featT", [T_steps, F, B], BF16, kind="ExternalInput")
    labpreT = nc.dram_tensor("labpreT", [T_steps, E, B], BF16, kind="ExternalInput")
    sT16 = nc.dram_tensor("sT16", [T_steps, B], BF16, kind="ExternalInput")
    out_ms = nc.dram_tensor("out_ms", [2, T_steps * B], F32, kind="ExternalOutput")

    RG = [list(range(NCORE))]

    with tile.TileContext(nc) as tc:
        with (
            tc.tile_pool(name="const", bufs=1) as constp,
            tc.tile_pool(name="state", bufs=2) as statep,
            tc.tile_pool(name="stream", bufs=3) as streamp,
            tc.tile_pool(name="work", bufs=2) as workp,
            tc.tile_pool(name="psum", bufs=2, space="PSUM") as psump,
            tc.tile_pool(name="dram", bufs=2, space="DRAM") as dramp,
        ):
            wall_sb = constp.tile([128, NWCH * G], BF16)
            for q in range(4):
                w = NWCH * G // 4
                nc.sync.dma_start(wall_sb[:, q * w:(q + 1) * w], wall[:, q * w:(q + 1) * w])
            wlab_sb = constp.tile([E, G], BF16)
            nc.sync.dma_start(wlab_sb[:], wlab[:])
            u_sb = constp.tile([1, G], BF16)
            nc.sync.dma_start(u_sb[:], uv[0:1, :])
            v_sb = constp.tile([1, G], BF16)
            nc.sync.dma_start(v_sb[:], uv[1:2, :])
            b01_sb = constp.tile([1, 2 * G], BF16)
            nc.sync.dma_start(b01_sb[:], bias01[:])
            pw0_sb = constp.tile([128, 2 * KH], BF16)
            nc.sync.dma_start(pw0_sb[:], pw0[:])
            pw1_sb = constp.tile([128, 2 * KH], BF16)
            nc.sync.dma_start(pw1_sb[:], pw1[:])
            msb_sb = constp.tile([2, 1], F32)
            nc.sync.dma_start(msb_sb[:], msbias[:])
            ones_sb = constp.tile([1, B], BF16)
            nc.any.memset(ones_sb[:], 1.0)
            ident = constp.tile([128, 128], BF16)
            make_identity(nc, ident[:])
            c0 = constp.tile([128, HS], F32)
            nc.vector.memset(c0[:], 0.0)
            c1 = constp.tile([128, HS], F32)
            nc.vector.memset(c1[:], 0.0)
            outbuf = constp.tile([2, T_steps * B], F32)

            def wchunk(c):
                return wall_sb[:, c * G:(c + 1) * G]

            def prefetch(t):
                fb = streamp.tile([128, KF * B], BF16, tag="feat", name="fb")
                nc.sync.dma_start(
                    fb[:].rearrange("p (k b) -> p k b", k=KF),
                    featT[t].rearrange("(k p) b -> p k b", p=128),
                )
                lb = streamp.tile([E, B], BF16, tag="lab", name="lb")
                nc.sync.dma_start(lb[:], labpreT[t])
                sb16 = streamp.tile([1, B], BF16, tag="s16", name="sb16")
                nc.sync.dma_start(sb16[:], sT16[t:t + 1, :])
                return fb, lb, sb16

            def lstm_nonlin(gp, c):
                sg = workp.tile([128, 3 * HS], F32, tag="sg", name="sg")
                nc.scalar.activation(sg[:], gp[:, 0:3 * HS], AF.Sigmoid)
                tg = workp.tile([128, HS], F32, tag="tg", name="tg")
                nc.scalar.activation(tg[:], gp[:, 3 * HS:4 * HS], AF.Tanh)
                t1 = workp.tile([128, HS], F32, tag="t1", name="t1")
                nc.vector.tensor_mul(t1[:], sg[:, HS:2 * HS], c[:])
                t2 = workp.tile([128, HS], F32, tag="t2", name="t2")
                nc.vector.tensor_mul(t2[:], sg[:, 0:HS], tg[:])
                nc.vector.tensor_add(c[:], t1[:], t2[:])
                tc_ = workp.tile([128, HS], F32, tag="tc", name="tc_")
                nc.scalar.activation(tc_[:], c[:], AF.Tanh)
                hb = workp.tile([128, HS], BF16, tag="hb", name="hb")
                nc.vector.tensor_mul(hb[:], sg[:, 2 * HS:3 * HS], tc_[:])
                return hb

            def transpose_out(hb):
                trp = psump.tile([128, 128], BF16, tag="tr", name="trp")
                nc.tensor.transpose(trp[:], hb[:], ident[:])
                hTs = workp.tile([128, 128], BF16, tag="hT", name="hTs")
                nc.vector.tensor_copy(hTs[:], trp[:])
                return hTs

            def exchange(hTs, tagbase):
                agin = dramp.tile([128, B], BF16, tag=tagbase + "in", name="agin")
                nc.sync.dma_start(agin[:], hTs[:])
                agout = dramp.tile([H, B], BF16, tag=tagbase + "out",
                                   addr_space="Shared", name="agout")
                if mode == "full":
                    nc.gpsimd.collective_compute(
                        "AllGather",
                        mybir.AluOpType.bypass,
                        replica_groups=RG,
                        ins=[agin.opt()],
                        outs=[agout.opt()],
                    )
                elif mode == "noag":
                    # timing diagnostic: keep the serial dependency chain but
                    # replace the ncfw collective with a core-local passthrough
                    nc.sync.dma_start(agout[0:128, :], agin[:])
                elif mode == "nocomm":
                    pass  # agout unwritten; land DMAs float free of the chain
                return agout

            def land(agout, tag):
                hT = statep.tile([128, H], BF16, tag=tag, name="hT" + tag)
                nc.sync.dma_start(
                    hT[:].rearrange("p (k b) -> p k b", k=KH),
                    agout.rearrange("(k p) b -> p k b", p=128),
                )
                return hT

            if mode == "rdma":
                hsems = [nc.alloc_semaphore("hsx0"), nc.alloc_semaphore("hsx1")]
                lsems = [nc.alloc_semaphore("lsx0"), nc.alloc_semaphore("lsx1")]
                pid = nc.gpsimd.partition_id()
                myoff = pid * B
                hpar = {
                    0: [constp.tile([128, H], BF16, name=f"h0Tp{p_}")
                        for p_ in range(2)],
                    1: [constp.tile([128, H], BF16, name=f"h1Tp{p_}")
                        for p_ in range(2)],
                }

                def exchange_rd(hb, which, t):
                    trp = psump.tile([128, 128], BF16, tag="tr", name="trp")
                    nc.tensor.transpose(trp[:], hb[:], ident[:])
                    hTs = workp.tile([128, 128], BF16, tag="hT", name="hTs")
                    # slot reuse: wait until our previous broadcast of this
                    # stream drained its source
                    nc.vector.tensor_copy(hTs[:], trp[:])._wait_ge(
                        lsems[which], 16 * t)
                    dst = hpar[which][t % 2]
                    nc.gpsimd.remote_dma_broadcast(
                        dst[:, bass.ds(myoff, B)],
                        hTs[:],
                        remote_sem=hsems[which],
                        local_sem=lsems[which],
                        rdests=[(0, k) for k in range(NCORE)],
                    )
                    nc.gpsimd.trigger_dma(count=None)
                    return dst

            # ---- preamble: pre-accumulate gates0 for t=0 ----
            fb_cur, lb_cur, s16_cur = prefetch(0)
            g0p = psump.tile([128, G], F32, tag="g0", name="g0p")
            nc.tensor.matmul(g0p[:], ones_sb[:], b01_sb[:, 0:G], start=True, stop=False)
            for k in range(KF):
                nc.tensor.matmul(g0p[:], fb_cur[:, ts(k, B)], wchunk(C_IH0 + k),
                                 start=False, stop=False)
            nc.tensor.matmul(g0p[:], lb_cur[:], wlab_sb[:], start=False, stop=True)

            h1T_prev = None
            obuf_prev = None

            for t in range(T_steps):
                last = t == T_steps - 1
                if t > 0:
                    # close gates0 with the mu-feedback rank-1 updates:
                    # g0 += u^T (s*mu) + v^T s
                    smu = workp.tile([1, B], BF16, tag="smu", name="smu")
                    nc.vector.tensor_mul(smu[:], s16_cur[:], obuf_prev[0:1, :])
                    nc.tensor.matmul(g0p[:], smu[:], u_sb[:], start=False, stop=False)
                    nc.tensor.matmul(g0p[:], s16_cur[:], v_sb[:], start=False, stop=True)

                hb0 = lstm_nonlin(g0p, c0)
                if mode == "rdma":
                    h0T = exchange_rd(hb0, 0, t)
                else:
                    hT0 = transpose_out(hb0)
                    ag0out = exchange(hT0, "x0")

                # hidden work while AG0 is in flight
                g1p = psump.tile([128, G], F32, tag="g1", name="g1p")
                nc.tensor.matmul(g1p[:], ones_sb[:], b01_sb[:, G:2 * G],
                                 start=True, stop=False)
                if h1T_prev is not None:
                    for k in range(KH):
                        mm = nc.tensor.matmul(g1p[:], h1T_prev[:, ts(k, 128)],
                                              wchunk(C_HH1 + k), start=False, stop=False)
                        if mode == "rdma" and k == 0:
                            mm._wait_ge(hsems[1], 16 * t)

                if mode != "rdma":
                    h0T = land(ag0out, "h0T")
                for k in range(KH):
                    mm = nc.tensor.matmul(g1p[:], h0T[:, ts(k, 128)], wchunk(C_IH1 + k),
                                          start=False, stop=(k == KH - 1))
                    if mode == "rdma" and k == 0:
                        mm._wait_ge(hsems[0], 16 * (t + 1))

                hb1 = lstm_nonlin(g1p, c1)
                if mode == "rdma":
                    h1T = exchange_rd(hb1, 1, t)
                else:
                    hT1 = transpose_out(hb1)
                    ag1out = exchange(hT1, "x1")

                # hidden work while AG1 is in flight
                msp = psump.tile([2, B], F32, tag="ms", name="msp")
                for k in range(KH):
                    mm = nc.tensor.matmul(msp[:], pw0_sb[:, ts(k, 2)], h0T[:, ts(k, 128)],
                                          start=(k == 0), stop=False)
                    if mode == "rdma" and k == 0:
                        mm._wait_ge(hsems[0], 16 * (t + 1))
                if not last:
                    fb_nxt, lb_nxt, s16_nxt = prefetch(t + 1)
                    g0p_nxt = psump.tile([128, G], F32, tag="g0", name="g0p")
                    nc.tensor.matmul(g0p_nxt[:], ones_sb[:], b01_sb[:, 0:G],
                                     start=True, stop=False)
                    for k in range(KF):
                        nc.tensor.matmul(g0p_nxt[:], fb_nxt[:, ts(k, B)],
                                         wchunk(C_IH0 + k), start=False, stop=False)
                    nc.tensor.matmul(g0p_nxt[:], lb_nxt[:], wlab_sb[:],
                                     start=False, stop=False)
                    for k in range(KH):
                        mm = nc.tensor.matmul(g0p_nxt[:], h0T[:, ts(k, 128)],
                                              wchunk(C_HH0 + k), start=False, stop=False)
                        if mode == "rdma" and k == 0:
                            mm._wait_ge(hsems[0], 16 * (t + 1))

                if mode != "rdma":
                    h1T = land(ag1out, "h1T")
                for k in range(KH):
                    mm = nc.tensor.matmul(msp[:], pw1_sb[:, ts(k, 2)], h1T[:, ts(k, 128)],
                                          start=False, stop=(k == KH - 1))
                    if mode == "rdma" and k == 0:
                        mm._wait_ge(hsems[1], 16 * (t + 1))

                obuf = outbuf[:, ts(t, B)]
                nc.vector.tensor_scalar_add(obuf[:], msp[:], msb_sb[:])

                h1T_prev = h1T
                obuf_prev = obuf
                if not last:
                    fb_cur, lb_cur, s16_cur = fb_nxt, lb_nxt, s16_nxt
                    g0p = g0p_nxt

            nc.sync.dma_start(out_ms[:], outbuf[:])

    nc.compile()
    return nc


def make_in_maps(inputs, T_steps=T):
    """Host-side preprocessing: full numpy inputs -> per-core input dicts."""
    feat = np.asarray(inputs["feat"], np.float32)
    lab = np.asarray(inputs["embedded_labels"], np.float32)
    mask = np.asarray(inputs["mask"])
    W_ih0 = np.asarray(inputs["W_ih0"], np.float32)
    W_hh0 = np.asarray(inputs["W_hh0"], np.float32)
    b_ih0 = np.asarray(inputs["b_ih0"], np.float32)
    b_hh0 = np.asarray(inputs["b_hh0"], np.float32)
    W_ih1 = np.asarray(inputs["W_ih1"], np.float32)
    W_hh1 = np.asarray(inputs["W_hh1"], np.float32)
    b_ih1 = np.asarray(inputs["b_ih1"], np.float32)
    b_hh1 = np.asarray(inputs["b_hh1"], np.float32)
    W_tgt = np.asarray(inputs["W_tgt"], np.float32)
    b_tgt = np.asarray(inputs["b_tgt"], np.float32)
    mu_w = np.asarray(inputs["mu_w"], np.float32)
    mu_b = np.asarray(inputs["mu_b"], np.float32)
    sig_w = np.asarray(inputs["sig_w"], np.float32)
    sig_b = np.asarray(inputs["sig_b"], np.float32)

    s = mask.T.astype(np.float32).copy()  # [T, B]
    s[0, :] = 0.0
    featT = np.ascontiguousarray(feat.transpose(1, 2, 0))[:T_steps]      # [T, F, B]
    labT = lab.transpose(1, 2, 0)                                        # [T, E, B]
    labpreT = np.ascontiguousarray(labT * (1.0 - s)[:, None, :])[:T_steps]
    sT = np.ascontiguousarray(s[:T_steps])

    def pwpack(vec_m, vec_s):
        arr = np.zeros((128, 2 * KH), np.float32)
        for k in range(KH):
            arr[:, 2 * k] = vec_m[128 * k:128 * (k + 1)]
            arr[:, 2 * k + 1] = vec_s[128 * k:128 * (k + 1)]
        return arr

    pw0 = pwpack(mu_w[0::2, 0], sig_w[0::2, 0])
    pw1 = pwpack(mu_w[1::2, 0], sig_w[1::2, 0])
    msbias = np.array([[mu_b[0]], [sig_b[0]]], np.float32)

    in_maps = []
    for j in range(NCORE):
        r0 = gate_rows(j)
        r1 = gate_rows(j)
        chunks = []
        for k in range(KF):
            chunks.append(W_ih0[r0][:, 128 * k:128 * (k + 1)].T)
        for k in range(KH):
            chunks.append(W_hh0[r0][:, 128 * k:128 * (k + 1)].T)
        for k in range(KH):
            chunks.append(W_ih1[r1][:, 128 * k:128 * (k + 1)].T)
        for k in range(KH):
            chunks.append(W_hh1[r1][:, 128 * k:128 * (k + 1)].T)
        wall = np.ascontiguousarray(np.concatenate(chunks, axis=1), dtype=np.float32)
        wlab_j = np.ascontiguousarray(W_ih0[r0][:, F:].T)           # [E, G]
        u_j = W_ih0[r0][:, F:] @ W_tgt[0]                           # [G]
        v_j = W_ih0[r0][:, F:] @ b_tgt                              # [G]
        uv_j = np.ascontiguousarray(np.stack([u_j, v_j]), dtype=np.float32)
        bias01_j = np.concatenate([(b_ih0 + b_hh0)[r0], (b_ih1 + b_hh1)[r1]])[None, :]
        bf = ml_dtypes.bfloat16
        in_maps.append({
            "wall": wall.astype(bf),
            "wlab": wlab_j.astype(bf),
            "uv": uv_j.astype(bf),
            "bias01": np.ascontiguousarray(bias01_j, dtype=np.float32).astype(bf),
            "pw0": pw0.astype(bf),
            "pw1": pw1.astype(bf),
            "msbias": msbias,
            "featT": featT.astype(bf),
            "labpreT": labpreT.astype(bf),
            "sT16": sT.astype(bf),
        })
    return in_maps


def postprocess(out_ms, T_steps=T):
    mu = out_ms[0].reshape(T_steps, B).T
    sigpre = out_ms[1].reshape(T_steps, B).T
    sig = np.logaddexp(0.0, sigpre)
    return np.stack([mu, sig], axis=-1).astype(np.float32)


_BUILT = {}


def kernel(**inputs):
    in_maps = make_in_maps(inputs)
    if T not in _BUILT:
        _BUILT[T] = build(T)
    nc = _BUILT[T]
    res = run_bass_kernel_spmd(nc, in_maps, list(range(NCORE)))
    return postprocess(res.results[0]["out_ms"])


# revision 25
# speedup vs baseline: 1.1148x; 1.1148x over previous
"""DeepAR (2-layer LSTM, H=1024, B=128, T=160) Trainium2 kernel.

Tensor-parallel over hidden units across 8 cores (each core owns a 128-unit
slice of each layer), full batch B=128 on every core (batch-major gates,
activation-stationary / weight-streaming matmuls, bf16 operands with fp32
PSUM).  Two AllGathers per step rebuild the hidden vectors.  The mu/sigma
projections ride INSIDE the h1 AllGather message: each core appends its
own-slice partial (pw0_j . h0_j + pw1_j . h1_j) as two extra columns, so
after the gather mu is one 4KB strided DMA + one DVE reduce — the 256KB h1
landing and the 16 projection matmuls sit off the feedback critical path.
The mu feedback itself is applied as two DVE scalar_tensor_tensor ops
(column-broadcast rank-1 updates u*(s*mu) + v'*s with mu_b folded into v'),
and softplus runs on the host (sigma is output-only).
"""

import sys

import ml_dtypes
import numpy as np

sys.path.insert(0, "/opt/trn_rl_repo")

import concourse.bass as bass
import concourse.mybir as mybir
import concourse.tile as tile
from concourse import bacc
from concourse.bass import ts
from concourse.bass_utils import run_bass_kernel_spmd
from concourse.masks import make_identity

F32 = mybir.dt.float32
BF16 = mybir.dt.bfloat16
AF = mybir.ActivationFunctionType
ALU = mybir.AluOpType

B = 128
T = 160
F = 512
E = 64
H = 1024
NCORE = 8
HS = H // NCORE          # hidden units per core per layer
G = 4 * HS               # gate rows per core (i|f|o|g blocks of 128)
KF = F // 128            # feat chunks
KH = H // 128            # hidden chunks
C_IH0, C_HH0, C_IH1, C_HH1 = 0, KF, KF + KH, KF + 2 * KH
NWCH = KF + 3 * KH       # 28 weight chunks of [128, G]
MW = B + 2               # h1 message width: 128 h columns + 2 partial columns


def gate_rows(j):
    """Gate rows (torch order i,f,g,o) owned by core j, reordered to i,f,o,g
    so that sigmoid gates occupy contiguous psum columns 0:384."""
    sl = np.arange(HS * j, HS * j + HS)
    return np.concatenate([0 * H + sl, 1 * H + sl, 3 * H + sl, 2 * H + sl])


def build(T_steps=T, mode="full"):
    nc = bacc.Bacc(num_devices=NCORE)

    wall = nc.dram_tensor("wall", [128, NWCH * G], BF16, kind="ExternalInput")
    wlab = nc.dram_tensor("wlab", [E, G], BF16, kind="ExternalInput")
    ubc = nc.dram_tensor("ubc", [128, G], BF16, kind="ExternalInput")
    vbc = nc.dram_tensor("vbc", [128, G], BF16, kind="ExternalInput")
    bias01 = nc.dram_tensor("bias01", [1, 2 * G], BF16, kind="ExternalInput")
    pwown = nc.dram_tensor("pwown", [128, 4], BF16, kind="ExternalInput")
    msb2 = nc.dram_tensor("msb2", [128, 2], F32, kind="ExternalInput")
    featT = nc.dram_tensor("featT", [T_steps, F, B], BF16, kind="ExternalInput")
    labpreT = nc.dram_tensor("labpreT", [T_steps, E, B], BF16, kind="ExternalInput")
    sT16 = nc.dram_tensor("sT16", [T_steps, B], BF16, kind="ExternalInput")
    out_ms = nc.dram_tensor("out_ms", [128, 2 * T_steps], F32, kind="ExternalOutput")

    RG = [list(range(NCORE))]

    with tile.TileContext(nc) as tc:
        with (
            tc.tile_pool(name="const", bufs=1) as constp,
            tc.tile_pool(name="state", bufs=2) as statep,
            tc.tile_pool(name="stream", bufs=3) as streamp,
            tc.tile_pool(name="work", bufs=2) as workp,
            tc.tile_pool(name="psum", bufs=2, space="PSUM") as psump,
            tc.tile_pool(name="dram", bufs=2, space="DRAM") as dramp,
        ):
            wall_sb = constp.tile([128, NWCH * G], BF16)
            for q in range(4):
                w = NWCH * G // 4
                nc.sync.dma_start(wall_sb[:, q * w:(q + 1) * w], wall[:, q * w:(q + 1) * w])
            wlab_sb = constp.tile([E, G], BF16)
            nc.sync.dma_start(wlab_sb[:], wlab[:])
            ubc_sb = constp.tile([128, G], BF16)
            nc.sync.dma_start(ubc_sb[:], ubc[:])
            vbc_sb = constp.tile([128, G], BF16)
            nc.sync.dma_start(vbc_sb[:], vbc[:])
            b01_sb = constp.tile([1, 2 * G], BF16)
            nc.sync.dma_start(b01_sb[:], bias01[:])
            pwown_sb = constp.tile([128, 4], BF16)
            nc.sync.dma_start(pwown_sb[:], pwown[:])
            msb2_sb = constp.tile([128, 2], F32)
            nc.sync.dma_start(msb2_sb[:], msb2[:])
            ones_sb = constp.tile([1, B], BF16)
            nc.any.memset(ones_sb[:], 1.0)
            ident = constp.tile([128, 128], BF16)
            make_identity(nc, ident[:])
            c0 = constp.tile([128, HS], F32)
            nc.vector.memset(c0[:], 0.0)
            c1 = constp.tile([128, HS], F32)
            nc.vector.memset(c1[:], 0.0)
            outbuf = constp.tile([128, 2 * T_steps], F32)

            def wchunk(c):
                return wall_sb[:, c * G:(c + 1) * G]

            def prefetch(t):
                fb = streamp.tile([128, KF * B], BF16, tag="feat", name="fb")
                nc.sync.dma_start(
                    fb[:].rearrange("p (k b) -> p k b", k=KF),
                    featT[t].rearrange("(k p) b -> p k b", p=128),
                )
                lb = streamp.tile([E, B], BF16, tag="lab", name="lb")
                nc.sync.dma_start(lb[:], labpreT[t])
                sc = streamp.tile([128, 1], BF16, tag="scol", name="sc")
                nc.sync.dma_start(sc[:], sT16[t:t + 1, :].rearrange("a b -> b a"))
                return fb, lb, sc

            def lstm_nonlin(gp, c):
                sg = workp.tile([128, 3 * HS], F32, tag="sg", name="sg")
                nc.scalar.activation(sg[:], gp[:, 0:3 * HS], AF.Sigmoid)
                tg = workp.tile([128, HS], F32, tag="tg", name="tg")
                nc.scalar.activation(tg[:], gp[:, 3 * HS:4 * HS], AF.Tanh)
                t1 = workp.tile([128, HS], F32, tag="t1", name="t1")
                nc.vector.tensor_mul(t1[:], sg[:, HS:2 * HS], c[:])
                t2 = workp.tile([128, HS], F32, tag="t2", name="t2")
                nc.vector.tensor_mul(t2[:], sg[:, 0:HS], tg[:])
                nc.vector.tensor_add(c[:], t1[:], t2[:])
                tc_ = workp.tile([128, HS], F32, tag="tc", name="tc_")
                nc.scalar.activation(tc_[:], c[:], AF.Tanh)
                hb = workp.tile([128, HS], BF16, tag="hb", name="hb")
                nc.vector.tensor_mul(hb[:], sg[:, 2 * HS:3 * HS], tc_[:])
                return hb

            def ag(agin, tagbase, width):
                agout = dramp.tile([H, width], BF16, tag=tagbase + "out",
                                   addr_space="Shared", name="agout")
                if mode == "full":
                    nc.gpsimd.collective_compute(
                        "AllGather",
                        mybir.AluOpType.bypass,
                        replica_groups=RG,
                        ins=[agin.opt()],
                        outs=[agout.opt()],
                    )
                elif mode == "noag":
                    nc.sync.dma_start(agout[0:128, :], agin[:])
                return agout

            # ---- preamble: pre-accumulate gates0 for t=0 ----
            fb_cur, lb_cur, sc_cur = prefetch(0)
            g0p = psump.tile([128, G], F32, tag="g0", name="g0p")
            nc.tensor.matmul(g0p[:], ones_sb[:], b01_sb[:, 0:G], start=True, stop=False)
            for k in range(KF):
                nc.tensor.matmul(g0p[:], fb_cur[:, ts(k, B)], wchunk(C_IH0 + k),
                                 start=False, stop=False)
            nc.tensor.matmul(g0p[:], lb_cur[:], wlab_sb[:], start=False, stop=True)

            h1T_prev = None
            sum_prev = None

            for t in range(T_steps):
                last = t == T_steps - 1
                if t > 0:
                    # mu feedback as DVE rank-1 updates:
                    # g0 += ubc*(s*mu_raw) + vbc*s   (mu_b folded into vbc)
                    smu = workp.tile([128, 1], F32, tag="smu", name="smu")
                    nc.vector.tensor_mul(smu[:], sc_cur[:], sum_prev[:, 0:1])
                    nc.vector.scalar_tensor_tensor(
                        g0p[:], ubc_sb[:], smu[:], g0p[:], ALU.mult, ALU.add)
                    nc.vector.scalar_tensor_tensor(
                        g0p[:], vbc_sb[:], sc_cur[:], g0p[:], ALU.mult, ALU.add)

                hb0 = lstm_nonlin(g0p, c0)
                trp0 = psump.tile([128, 128], BF16, tag="tr", name="trp0")
                nc.tensor.transpose(trp0[:], hb0[:], ident[:])
                hTs0 = workp.tile([128, 128], BF16, tag="hT", name="hTs0")
                nc.vector.tensor_copy(hTs0[:], trp0[:])
                agin0 = dramp.tile([128, B], BF16, tag="x0in", name="agin0")
                nc.sync.dma_start(agin0[:], hTs0[:])
                ag0out = ag(agin0, "x0", B)

                # hidden work while AG0 is in flight
                g1p = psump.tile([128, G], F32, tag="g1", name="g1p")
                nc.tensor.matmul(g1p[:], ones_sb[:], b01_sb[:, G:2 * G],
                                 start=True, stop=False)
                if h1T_prev is not None:
                    for k in range(KH):
                        nc.tensor.matmul(g1p[:], h1T_prev[:, ts(k, 128)],
                                         wchunk(C_HH1 + k), start=False, stop=False)

                h0T = statep.tile([128, H], BF16, tag="h0T", name="hTh0T")
                nc.sync.dma_start(
                    h0T[:].rearrange("p (k b) -> p k b", k=KH),
                    ag0out.rearrange("(k p) b -> p k b", p=128),
                )
                for k in range(KH):
                    nc.tensor.matmul(g1p[:], h0T[:, ts(k, 128)], wchunk(C_IH1 + k),
                                     start=False, stop=(k == KH - 1))

                hb1 = lstm_nonlin(g1p, c1)
                # h1 message: [h1_j^T | own-slice mu/sig partial]
                trp1 = psump.tile([128, 128], BF16, tag="tr", name="trp1")
                nc.tensor.transpose(trp1[:], hb1[:], ident[:])
                msg = workp.tile([128, MW], BF16, tag="msg", name="msg")
                nc.vector.tensor_copy(msg[:, 0:B], trp1[:])
                part = psump.tile([128, 2], F32, tag="ms", name="part")
                nc.tensor.matmul(part[:], hTs0[:], pwown_sb[:, 0:2],
                                 start=True, stop=False)
                nc.tensor.matmul(part[:], msg[:, 0:B], pwown_sb[:, 2:4],
                                 start=False, stop=True)
                nc.vector.tensor_copy(msg[:, B:MW], part[:])
                agin1 = dramp.tile([128, MW], BF16, tag="x1in", name="agin1")
                nc.sync.dma_start(agin1[:], msg[:])
                ag1out = ag(agin1, "x1", MW)

                # hidden work while AG1 is in flight
                if not last:
                    fb_nxt, lb_nxt, sc_nxt = prefetch(t + 1)
                    g0p_nxt = psump.tile([128, G], F32, tag="g0", name="g0p")
                    nc.tensor.matmul(g0p_nxt[:], ones_sb[:], b01_sb[:, 0:G],
                                     start=True, stop=False)
                    for k in range(KF):
                        nc.tensor.matmul(g0p_nxt[:], fb_nxt[:, ts(k, B)],
                                         wchunk(C_IH0 + k), start=False, stop=False)
                    nc.tensor.matmul(g0p_nxt[:], lb_nxt[:], wlab_sb[:],
                                     start=False, stop=False)
                    for k in range(KH):
                        nc.tensor.matmul(g0p_nxt[:], h0T[:, ts(k, 128)],
                                         wchunk(C_HH0 + k), start=False,
                                         stop=(k == KH - 1))

                # mu/sigma: gather the 8 appended partials (4KB) and reduce —
                # the 256KB h1 landing below is NOT on this path
                psb = workp.tile([128, 2 * NCORE], BF16, tag="psb", name="psb")
                nc.sync.dma_start(
                    psb[:].rearrange("p (j c) -> p j c", c=2),
                    ag1out.rearrange("(j p) c -> p j c", p=128)[:, :, B:MW],
                )
                summed = workp.tile([128, 2], F32, tag="summed", name="summed")
                nc.vector.tensor_reduce(
                    summed[:],
                    psb[:].rearrange("p (j c) -> p c j", c=2),
                    mybir.AxisListType.X,
                    ALU.add,
                )
                nc.vector.tensor_add(outbuf[:, 2 * t:2 * t + 2], summed[:], msb2_sb[:])

                h1T = statep.tile([128, H], BF16, tag="h1T", name="hTh1T")
                nc.sync.dma_start(
                    h1T[:].rearrange("p (k b) -> p k b", k=KH),
                    ag1out[:, 0:B].rearrange("(k p) b -> p k b", p=128),
                )

                h1T_prev = h1T
                sum_prev = summed
                if not last:
                    fb_cur, lb_cur, sc_cur = fb_nxt, lb_nxt, sc_nxt
                    g0p = g0p_nxt

            nc.sync.dma_start(out_ms[:], outbuf[:])

    nc.compile()
    return nc


def make_in_maps(inputs, T_steps=T):
    """Host-side preprocessing: full numpy inputs -> per-core input dicts."""
    feat = np.asarray(inputs["feat"], np.float32)
    lab = np.asarray(inputs["embedded_labels"], np.float32)
    mask = np.asarray(inputs["mask"])
    W_ih0 = np.asarray(inputs["W_ih0"], np.float32)
    W_hh0 = np.asarray(inputs["W_hh0"], np.float32)
    b_ih0 = np.asarray(inputs["b_ih0"], np.float32)
    b_hh0 = np.asarray(inputs["b_hh0"], np.float32)
    W_ih1 = np.asarray(inputs["W_ih1"], np.float32)
    W_hh1 = np.asarray(inputs["W_hh1"], np.float32)
    b_ih1 = np.asarray(inputs["b_ih1"], np.float32)
    b_hh1 = np.asarray(inputs["b_hh1"], np.float32)
    W_tgt = np.asarray(inputs["W_tgt"], np.float32)
    b_tgt = np.asarray(inputs["b_tgt"], np.float32)
    mu_w = np.asarray(inputs["mu_w"], np.float32)
    mu_b = np.asarray(inputs["mu_b"], np.float32)
    sig_w = np.asarray(inputs["sig_w"], np.float32)
    sig_b = np.asarray(inputs["sig_b"], np.float32)

    s = mask.T.astype(np.float32).copy()  # [T, B]
    s[0, :] = 0.0
    featT = np.ascontiguousarray(feat.transpose(1, 2, 0))[:T_steps]      # [T, F, B]
    labT = lab.transpose(1, 2, 0)                                        # [T, E, B]
    labpreT = np.ascontiguousarray(labT * (1.0 - s)[:, None, :])[:T_steps]
    sT = np.ascontiguousarray(s[:T_steps])

    mu_e, mu_o = mu_w[0::2, 0], mu_w[1::2, 0]
    sig_e, sig_o = sig_w[0::2, 0], sig_w[1::2, 0]
    msb2 = np.tile(np.array([[mu_b[0], sig_b[0]]], np.float32), (128, 1))

    bf = ml_dtypes.bfloat16
    in_maps = []
    for j in range(NCORE):
        r0 = gate_rows(j)
        r1 = gate_rows(j)
        chunks = []
        for k in range(KF):
            chunks.append(W_ih0[r0][:, 128 * k:128 * (k + 1)].T)
        for k in range(KH):
            chunks.append(W_hh0[r0][:, 128 * k:128 * (k + 1)].T)
        for k in range(KH):
            chunks.append(W_ih1[r1][:, 128 * k:128 * (k + 1)].T)
        for k in range(KH):
            chunks.append(W_hh1[r1][:, 128 * k:128 * (k + 1)].T)
        wall = np.ascontiguousarray(np.concatenate(chunks, axis=1), dtype=np.float32)
        wlab_j = np.ascontiguousarray(W_ih0[r0][:, F:].T)           # [E, G]
        u_j = W_ih0[r0][:, F:] @ W_tgt[0]                           # [G]
        v_j = W_ih0[r0][:, F:] @ b_tgt + float(mu_b[0]) * u_j       # mu_b folded
        ubc_j = np.tile(u_j[None, :], (128, 1))
        vbc_j = np.tile(v_j[None, :], (128, 1))
        sl = slice(HS * j, HS * j + HS)
        pwown_j = np.stack([mu_e[sl], sig_e[sl], mu_o[sl], sig_o[sl]], axis=1)
        bias01_j = np.concatenate([(b_ih0 + b_hh0)[r0], (b_ih1 + b_hh1)[r1]])[None, :]
        in_maps.append({
            "wall": wall.astype(bf),
            "wlab": wlab_j.astype(bf),
            "ubc": ubc_j.astype(bf),
            "vbc": vbc_j.astype(bf),
            "bias01": np.ascontiguousarray(bias01_j, dtype=np.float32).astype(bf),
            "pwown": np.ascontiguousarray(pwown_j, dtype=np.float32).astype(bf),
            "msb2": msb2,
            "featT": featT.astype(bf),
            "labpreT": labpreT.astype(bf),
            "sT16": sT.astype(bf),
        })
    return in_maps


def postprocess(out_ms, T_steps=T):
    out = out_ms.reshape(128, T_steps, 2)
    mu = out[:, :, 0]
    sig = np.logaddexp(0.0, out[:, :, 1])
    return np.stack([mu, sig], axis=-1).astype(np.float32)


_BUILT = {}


def kernel(**inputs):
    in_maps = make_in_maps(inputs)
    if T not in _BUILT:
        _BUILT[T] = build(T)
    nc = _BUILT[T]
    res = run_bass_kernel_spmd(nc, in_maps, list(range(NCORE)))
    return postprocess(res.results[0]["out_ms"])


# revision 26
# speedup vs baseline: 1.1274x; 1.0113x over previous
"""DeepAR (2-layer LSTM, H=1024, B=128, T=160) Trainium2 kernel.

Tensor-parallel over hidden units across 8 cores (each core owns a 128-unit
slice of each layer), full batch B=128 on every core (batch-major gates,
activation-stationary / weight-streaming matmuls, bf16 operands with fp32
PSUM).  Two AllGathers per step rebuild the hidden vectors.  The mu/sigma
projections ride INSIDE the h1 AllGather message: each core appends its
own-slice partial (pw0_j . h0_j + pw1_j . h1_j) as two extra columns, so
after the gather mu is one 4KB strided DMA + one DVE reduce — the 256KB h1
landing and the 16 projection matmuls sit off the feedback critical path.
The mu feedback itself is applied as two DVE scalar_tensor_tensor ops
(column-broadcast rank-1 updates u*(s*mu) + v'*s with mu_b folded into v'),
and softplus runs on the host (sigma is output-only).
"""

import sys

import ml_dtypes
import numpy as np

sys.path.insert(0, "/opt/trn_rl_repo")

import concourse.bass as bass
import concourse.mybir as mybir
import concourse.tile as tile
from concourse import bacc
from concourse.bass import ts
from concourse.bass_utils import run_bass_kernel_spmd
from concourse.masks import make_identity

F32 = mybir.dt.float32
BF16 = mybir.dt.bfloat16
AF = mybir.ActivationFunctionType
ALU = mybir.AluOpType

B = 128
T = 160
F = 512
E = 64
H = 1024
NCORE = 8
HS = H // NCORE          # hidden units per core per layer
G = 4 * HS               # gate rows per core (i|f|o|g blocks of 128)
KF = F // 128            # feat chunks
KH = H // 128            # hidden chunks
C_IH0, C_HH0, C_IH1, C_HH1 = 0, KF, KF + KH, KF + 2 * KH
NWCH = KF + 3 * KH       # 28 weight chunks of [128, G]
MW = B + 2               # h1 message width: 128 h columns + 2 partial columns


def gate_rows(j):
    """Gate rows (torch order i,f,g,o) owned by core j, reordered to i,f,o,g
    so that sigmoid gates occupy contiguous psum columns 0:384."""
    sl = np.arange(HS * j, HS * j + HS)
    return np.concatenate([0 * H + sl, 1 * H + sl, 3 * H + sl, 2 * H + sl])


def build(T_steps=T, mode="full"):
    nc = bacc.Bacc(num_devices=NCORE)

    wall = nc.dram_tensor("wall", [128, NWCH * G], BF16, kind="ExternalInput")
    wlab = nc.dram_tensor("wlab", [E, G], BF16, kind="ExternalInput")
    ubc = nc.dram_tensor("ubc", [128, G], BF16, kind="ExternalInput")
    vbc = nc.dram_tensor("vbc", [128, G], BF16, kind="ExternalInput")
    bias01 = nc.dram_tensor("bias01", [1, 2 * G], BF16, kind="ExternalInput")
    pwown = nc.dram_tensor("pwown", [128, 4], BF16, kind="ExternalInput")
    msb2 = nc.dram_tensor("msb2", [128, 2], F32, kind="ExternalInput")
    featT = nc.dram_tensor("featT", [T_steps, F, B], BF16, kind="ExternalInput")
    labpreT = nc.dram_tensor("labpreT", [T_steps, E, B], BF16, kind="ExternalInput")
    sT16 = nc.dram_tensor("sT16", [T_steps, B], BF16, kind="ExternalInput")
    out_ms = nc.dram_tensor("out_ms", [128, 2 * T_steps], F32, kind="ExternalOutput")

    RG = [list(range(NCORE))]

    with tile.TileContext(nc) as tc:
        with (
            tc.tile_pool(name="const", bufs=1) as constp,
            tc.tile_pool(name="state", bufs=2) as statep,
            tc.tile_pool(name="stream", bufs=3) as streamp,
            tc.tile_pool(name="work", bufs=2) as workp,
            tc.tile_pool(name="psum", bufs=2, space="PSUM") as psump,
            tc.tile_pool(name="dram", bufs=2, space="DRAM") as dramp,
        ):
            wall_sb = constp.tile([128, NWCH * G], BF16)
            for q in range(4):
                w = NWCH * G // 4
                nc.sync.dma_start(wall_sb[:, q * w:(q + 1) * w], wall[:, q * w:(q + 1) * w])
            wlab_sb = constp.tile([E, G], BF16)
            nc.sync.dma_start(wlab_sb[:], wlab[:])
            ubc_sb = constp.tile([128, G], BF16)
            nc.sync.dma_start(ubc_sb[:], ubc[:])
            vbc_sb = constp.tile([128, G], BF16)
            nc.sync.dma_start(vbc_sb[:], vbc[:])
            b01_sb = constp.tile([1, 2 * G], BF16)
            nc.sync.dma_start(b01_sb[:], bias01[:])
            pwown_sb = constp.tile([128, 4], BF16)
            nc.sync.dma_start(pwown_sb[:], pwown[:])
            msb2_sb = constp.tile([128, 2], F32)
            nc.sync.dma_start(msb2_sb[:], msb2[:])
            ones_sb = constp.tile([1, B], BF16)
            nc.any.memset(ones_sb[:], 1.0)
            ident = constp.tile([128, 128], BF16)
            make_identity(nc, ident[:])
            c0 = constp.tile([128, HS], F32)
            nc.vector.memset(c0[:], 0.0)
            c1 = constp.tile([128, HS], F32)
            nc.vector.memset(c1[:], 0.0)
            outbuf = constp.tile([128, 2 * T_steps], F32)

            def wchunk(c):
                return wall_sb[:, c * G:(c + 1) * G]

            def prefetch(t):
                fb = streamp.tile([128, KF * B], BF16, tag="feat", name="fb")
                nc.sync.dma_start(
                    fb[:].rearrange("p (k b) -> p k b", k=KF),
                    featT[t].rearrange("(k p) b -> p k b", p=128),
                )
                lb = streamp.tile([E, B], BF16, tag="lab", name="lb")
                nc.sync.dma_start(lb[:], labpreT[t])
                sc = streamp.tile([128, 1], BF16, tag="scol", name="sc")
                nc.sync.dma_start(sc[:], sT16[t:t + 1, :].rearrange("a b -> b a"))
                return fb, lb, sc

            def lstm_nonlin(gp, c):
                sg = workp.tile([128, 3 * HS], F32, tag="sg", name="sg")
                nc.scalar.activation(sg[:], gp[:, 0:3 * HS], AF.Sigmoid)
                tg = workp.tile([128, HS], F32, tag="tg", name="tg")
                nc.scalar.activation(tg[:], gp[:, 3 * HS:4 * HS], AF.Tanh)
                t1 = workp.tile([128, HS], F32, tag="t1", name="t1")
                nc.vector.tensor_mul(t1[:], sg[:, HS:2 * HS], c[:])
                t2 = workp.tile([128, HS], F32, tag="t2", name="t2")
                nc.vector.tensor_mul(t2[:], sg[:, 0:HS], tg[:])
                nc.vector.tensor_add(c[:], t1[:], t2[:])
                tc_ = workp.tile([128, HS], F32, tag="tc", name="tc_")
                nc.scalar.activation(tc_[:], c[:], AF.Tanh)
                hb = workp.tile([128, HS], BF16, tag="hb", name="hb")
                nc.vector.tensor_mul(hb[:], sg[:, 2 * HS:3 * HS], tc_[:])
                return hb

            def ag(agin, tagbase, width):
                agout = dramp.tile([H, width], BF16, tag=tagbase + "out",
                                   addr_space="Shared", name="agout")
                if mode == "full":
                    nc.gpsimd.collective_compute(
                        "AllGather",
                        mybir.AluOpType.bypass,
                        replica_groups=RG,
                        ins=[agin.opt()],
                        outs=[agout.opt()],
                    )
                elif mode == "noag":
                    nc.sync.dma_start(agout[0:128, :], agin[:])
                return agout

            # ---- preamble: pre-accumulate gates0 for t=0 ----
            fb_cur, lb_cur, sc_cur = prefetch(0)
            g0p = psump.tile([128, G], F32, tag="g0", name="g0p")
            nc.tensor.matmul(g0p[:], ones_sb[:], b01_sb[:, 0:G], start=True, stop=False)
            for k in range(KF):
                nc.tensor.matmul(g0p[:], fb_cur[:, ts(k, B)], wchunk(C_IH0 + k),
                                 start=False, stop=False)
            nc.tensor.matmul(g0p[:], lb_cur[:], wlab_sb[:], start=False, stop=True)

            h1T_prev = None
            sum_prev = None

            for t in range(T_steps):
                last = t == T_steps - 1
                if t > 0:
                    # mu feedback as DVE rank-1 updates:
                    # g0 += ubc*(s*mu_raw) + vbc*s   (mu_b folded into vbc)
                    smu = workp.tile([128, 1], F32, tag="smu", name="smu")
                    nc.vector.tensor_mul(smu[:], sc_cur[:], sum_prev[:, 0:1])
                    nc.vector.scalar_tensor_tensor(
                        g0p[:], ubc_sb[:], smu[:], g0p[:], ALU.mult, ALU.add)
                    nc.vector.scalar_tensor_tensor(
                        g0p[:], vbc_sb[:], sc_cur[:], g0p[:], ALU.mult, ALU.add)

                hb0 = lstm_nonlin(g0p, c0)
                trp0 = psump.tile([128, 128], BF16, tag="tr", name="trp0")
                nc.tensor.transpose(trp0[:], hb0[:], ident[:])
                hTs0 = workp.tile([128, 128], BF16, tag="hT", name="hTs0")
                nc.vector.tensor_copy(hTs0[:], trp0[:])
                agin0 = dramp.tile([128, B], BF16, tag="x0in", name="agin0")
                nc.sync.dma_start(agin0[:], hTs0[:])
                ag0out = ag(agin0, "x0", B)

                # hidden work while AG0 is in flight
                g1p = psump.tile([128, G], F32, tag="g1", name="g1p")
                nc.tensor.matmul(g1p[:], ones_sb[:], b01_sb[:, G:2 * G],
                                 start=True, stop=False)
                if h1T_prev is not None:
                    for k in range(KH):
                        nc.tensor.matmul(g1p[:], h1T_prev[:, ts(k, 128)],
                                         wchunk(C_HH1 + k), start=False, stop=False)

                h0T = statep.tile([128, H], BF16, tag="h0T", name="hTh0T")
                # land in two halves so the first Wih1 matmuls overlap the
                # second half of the 256KB landing DMA
                KH2 = KH // 2
                ag0v = ag0out.rearrange("(k p) b -> p k b", p=128)
                h0Tv = h0T[:].rearrange("p (k b) -> p k b", k=KH)
                nc.sync.dma_start(h0Tv[:, 0:KH2, :], ag0v[:, 0:KH2, :])
                nc.sync.dma_start(h0Tv[:, KH2:KH, :], ag0v[:, KH2:KH, :])
                for k in range(KH):
                    nc.tensor.matmul(g1p[:], h0T[:, ts(k, 128)], wchunk(C_IH1 + k),
                                     start=False, stop=(k == KH - 1))

                hb1 = lstm_nonlin(g1p, c1)
                # h1 message: [h1_j^T | own-slice mu/sig partial]
                trp1 = psump.tile([128, 128], BF16, tag="tr", name="trp1")
                nc.tensor.transpose(trp1[:], hb1[:], ident[:])
                msg = workp.tile([128, MW], BF16, tag="msg", name="msg")
                nc.vector.tensor_copy(msg[:, 0:B], trp1[:])
                part = psump.tile([128, 2], F32, tag="ms", name="part")
                nc.tensor.matmul(part[:], hTs0[:], pwown_sb[:, 0:2],
                                 start=True, stop=False)
                nc.tensor.matmul(part[:], msg[:, 0:B], pwown_sb[:, 2:4],
                                 start=False, stop=True)
                nc.vector.tensor_copy(msg[:, B:MW], part[:])
                agin1 = dramp.tile([128, MW], BF16, tag="x1in", name="agin1")
                nc.sync.dma_start(agin1[:], msg[:])
                ag1out = ag(agin1, "x1", MW)

                # hidden work while AG1 is in flight
                if not last:
                    fb_nxt, lb_nxt, sc_nxt = prefetch(t + 1)
                    g0p_nxt = psump.tile([128, G], F32, tag="g0", name="g0p")
                    nc.tensor.matmul(g0p_nxt[:], ones_sb[:], b01_sb[:, 0:G],
                                     start=True, stop=False)
                    for k in range(KF):
                        nc.tensor.matmul(g0p_nxt[:], fb_nxt[:, ts(k, B)],
                                         wchunk(C_IH0 + k), start=False, stop=False)
                    nc.tensor.matmul(g0p_nxt[:], lb_nxt[:], wlab_sb[:],
                                     start=False, stop=False)
                    for k in range(KH):
                        nc.tensor.matmul(g0p_nxt[:], h0T[:, ts(k, 128)],
                                         wchunk(C_HH0 + k), start=False,
                                         stop=(k == KH - 1))

                # mu/sigma: gather the 8 appended partials (4KB) and reduce —
                # the 256KB h1 landing below is NOT on this path
                psb = workp.tile([128, 2 * NCORE], BF16, tag="psb", name="psb")
                nc.sync.dma_start(
                    psb[:].rearrange("p (j c) -> p j c", c=2),
                    ag1out.rearrange("(j p) c -> p j c", p=128)[:, :, B:MW],
                )
                summed = workp.tile([128, 2], F32, tag="summed", name="summed")
                nc.vector.tensor_reduce(
                    summed[:],
                    psb[:].rearrange("p (j c) -> p c j", c=2),
                    mybir.AxisListType.X,
                    ALU.add,
                )
                nc.vector.tensor_add(outbuf[:, 2 * t:2 * t + 2], summed[:], msb2_sb[:])

                h1T = statep.tile([128, H], BF16, tag="h1T", name="hTh1T")
                nc.sync.dma_start(
                    h1T[:].rearrange("p (k b) -> p k b", k=KH),
                    ag1out[:, 0:B].rearrange("(k p) b -> p k b", p=128),
                )

                h1T_prev = h1T
                sum_prev = summed
                if not last:
                    fb_cur, lb_cur, sc_cur = fb_nxt, lb_nxt, sc_nxt
                    g0p = g0p_nxt

            nc.sync.dma_start(out_ms[:], outbuf[:])

    nc.compile()
    return nc


def make_in_maps(inputs, T_steps=T):
    """Host-side preprocessing: full numpy inputs -> per-core input dicts."""
    feat = np.asarray(inputs["feat"], np.float32)
    lab = np.asarray(inputs["embedded_labels"], np.float32)
    mask = np.asarray(inputs["mask"])
    W_ih0 = np.asarray(inputs["W_ih0"], np.float32)
    W_hh0 = np.asarray(inputs["W_hh0"], np.float32)
    b_ih0 = np.asarray(inputs["b_ih0"], np.float32)
    b_hh0 = np.asarray(inputs["b_hh0"], np.float32)
    W_ih1 = np.asarray(inputs["W_ih1"], np.float32)
    W_hh1 = np.asarray(inputs["W_hh1"], np.float32)
    b_ih1 = np.asarray(inputs["b_ih1"], np.float32)
    b_hh1 = np.asarray(inputs["b_hh1"], np.float32)
    W_tgt = np.asarray(inputs["W_tgt"], np.float32)
    b_tgt = np.asarray(inputs["b_tgt"], np.float32)
    mu_w = np.asarray(inputs["mu_w"], np.float32)
    mu_b = np.asarray(inputs["mu_b"], np.float32)
    sig_w = np.asarray(inputs["sig_w"], np.float32)
    sig_b = np.asarray(inputs["sig_b"], np.float32)

    s = mask.T.astype(np.float32).copy()  # [T, B]
    s[0, :] = 0.0
    featT = np.ascontiguousarray(feat.transpose(1, 2, 0))[:T_steps]      # [T, F, B]
    labT = lab.transpose(1, 2, 0)                                        # [T, E, B]
    labpreT = np.ascontiguousarray(labT * (1.0 - s)[:, None, :])[:T_steps]
    sT = np.ascontiguousarray(s[:T_steps])

    mu_e, mu_o = mu_w[0::2, 0], mu_w[1::2, 0]
    sig_e, sig_o = sig_w[0::2, 0], sig_w[1::2, 0]
    msb2 = np.tile(np.array([[mu_b[0], sig_b[0]]], np.float32), (128, 1))

    bf = ml_dtypes.bfloat16
    in_maps = []
    for j in range(NCORE):
        r0 = gate_rows(j)
        r1 = gate_rows(j)
        chunks = []
        for k in range(KF):
            chunks.append(W_ih0[r0][:, 128 * k:128 * (k + 1)].T)
        for k in range(KH):
            chunks.append(W_hh0[r0][:, 128 * k:128 * (k + 1)].T)
        for k in range(KH):
            chunks.append(W_ih1[r1][:, 128 * k:128 * (k + 1)].T)
        for k in range(KH):
            chunks.append(W_hh1[r1][:, 128 * k:128 * (k + 1)].T)
        wall = np.ascontiguousarray(np.concatenate(chunks, axis=1), dtype=np.float32)
        wlab_j = np.ascontiguousarray(W_ih0[r0][:, F:].T)           # [E, G]
        u_j = W_ih0[r0][:, F:] @ W_tgt[0]                           # [G]
        v_j = W_ih0[r0][:, F:] @ b_tgt + float(mu_b[0]) * u_j       # mu_b folded
        ubc_j = np.tile(u_j[None, :], (128, 1))
        vbc_j = np.tile(v_j[None, :], (128, 1))
        sl = slice(HS * j, HS * j + HS)
        pwown_j = np.stack([mu_e[sl], sig_e[sl], mu_o[sl], sig_o[sl]], axis=1)
        bias01_j = np.concatenate([(b_ih0 + b_hh0)[r0], (b_ih1 + b_hh1)[r1]])[None, :]
        in_maps.append({
            "wall": wall.astype(bf),
            "wlab": wlab_j.astype(bf),
            "ubc": ubc_j.astype(bf),
            "vbc": vbc_j.astype(bf),
            "bias01": np.ascontiguousarray(bias01_j, dtype=np.float32).astype(bf),
            "pwown": np.ascontiguousarray(pwown_j, dtype=np.float32).astype(bf),
            "msb2": msb2,
            "featT": featT.astype(bf),
            "labpreT": labpreT.astype(bf),
            "sT16": sT.astype(bf),
        })
    return in_maps


def postprocess(out_ms, T_steps=T):
    out = out_ms.reshape(128, T_steps, 2)
    mu = out[:, :, 0]
    sig = np.logaddexp(0.0, out[:, :, 1])
    return np.stack([mu, sig], axis=-1).astype(np.float32)


_BUILT = {}


def kernel(**inputs):
    in_maps = make_in_maps(inputs)
    if T not in _BUILT:
        _BUILT[T] = build(T)
    nc = _BUILT[T]
    res = run_bass_kernel_spmd(nc, in_maps, list(range(NCORE)))
    return postprocess(res.results[0]["out_ms"])


# revision 27
# speedup vs baseline: 1.1343x; 1.0061x over previous
"""DeepAR (2-layer LSTM, H=1024, B=128, T=160) Trainium2 kernel.

Tensor-parallel over hidden units across 8 cores (each core owns a 128-unit
slice of each layer), full batch B=128 on every core (batch-major gates,
activation-stationary / weight-streaming matmuls, bf16 operands with fp32
PSUM).  Two AllGathers per step rebuild the hidden vectors.  The mu/sigma
projections ride INSIDE the h1 AllGather message: each core appends its
own-slice partial (pw0_j . h0_j + pw1_j . h1_j) as two extra columns, so
after the gather mu is one 4KB strided DMA + one DVE reduce — the 256KB h1
landing and the 16 projection matmuls sit off the feedback critical path.
The mu feedback itself is applied as two DVE scalar_tensor_tensor ops
(column-broadcast rank-1 updates u*(s*mu) + v'*s with mu_b folded into v'),
and softplus runs on the host (sigma is output-only).
"""

import sys

import ml_dtypes
import numpy as np

sys.path.insert(0, "/opt/trn_rl_repo")

import concourse.bass as bass
import concourse.mybir as mybir
import concourse.tile as tile
from concourse import bacc
from concourse.bass import ts
from concourse.bass_utils import run_bass_kernel_spmd
from concourse.masks import make_identity

F32 = mybir.dt.float32
BF16 = mybir.dt.bfloat16
AF = mybir.ActivationFunctionType
ALU = mybir.AluOpType

B = 128
T = 160
F = 512
E = 64
H = 1024
NCORE = 8
HS = H // NCORE          # hidden units per core per layer
G = 4 * HS               # gate rows per core (i|f|o|g blocks of 128)
KF = F // 128            # feat chunks
KH = H // 128            # hidden chunks
C_IH0, C_HH0, C_IH1, C_HH1 = 0, KF, KF + KH, KF + 2 * KH
NWCH = KF + 3 * KH       # 28 weight chunks of [128, G]
MW = B + 2               # h1 message width: 128 h columns + 2 partial columns


def gate_rows(j):
    """Gate rows (torch order i,f,g,o) owned by core j, reordered to i,f,o,g
    so that sigmoid gates occupy contiguous psum columns 0:384."""
    sl = np.arange(HS * j, HS * j + HS)
    return np.concatenate([0 * H + sl, 1 * H + sl, 3 * H + sl, 2 * H + sl])


def build(T_steps=T, mode="full"):
    nc = bacc.Bacc(num_devices=NCORE)

    wall = nc.dram_tensor("wall", [128, NWCH * G], BF16, kind="ExternalInput")
    wlab = nc.dram_tensor("wlab", [E, G], BF16, kind="ExternalInput")
    ubc = nc.dram_tensor("ubc", [128, G], BF16, kind="ExternalInput")
    vbc = nc.dram_tensor("vbc", [128, G], BF16, kind="ExternalInput")
    bias01 = nc.dram_tensor("bias01", [1, 2 * G], BF16, kind="ExternalInput")
    pwown = nc.dram_tensor("pwown", [128, 4], BF16, kind="ExternalInput")
    msb2 = nc.dram_tensor("msb2", [128, 2], F32, kind="ExternalInput")
    featT = nc.dram_tensor("featT", [T_steps, F, B], BF16, kind="ExternalInput")
    labpreT = nc.dram_tensor("labpreT", [T_steps, E, B], BF16, kind="ExternalInput")
    sT16 = nc.dram_tensor("sT16", [T_steps, B], BF16, kind="ExternalInput")
    out_ms = nc.dram_tensor("out_ms", [128, 2 * T_steps], F32, kind="ExternalOutput")

    RG = [list(range(NCORE))]

    with tile.TileContext(nc) as tc:
        with (
            tc.tile_pool(name="const", bufs=1) as constp,
            tc.tile_pool(name="state", bufs=2) as statep,
            tc.tile_pool(name="stream", bufs=3) as streamp,
            tc.tile_pool(name="work", bufs=2) as workp,
            tc.tile_pool(name="psum", bufs=2, space="PSUM") as psump,
            tc.tile_pool(name="dram", bufs=2, space="DRAM") as dramp,
        ):
            wall_sb = constp.tile([128, NWCH * G], BF16)
            for q in range(4):
                w = NWCH * G // 4
                nc.sync.dma_start(wall_sb[:, q * w:(q + 1) * w], wall[:, q * w:(q + 1) * w])
            wlab_sb = constp.tile([E, G], BF16)
            nc.sync.dma_start(wlab_sb[:], wlab[:])
            ubc_sb = constp.tile([128, G], BF16)
            nc.sync.dma_start(ubc_sb[:], ubc[:])
            vbc_sb = constp.tile([128, G], BF16)
            nc.sync.dma_start(vbc_sb[:], vbc[:])
            b01_sb = constp.tile([1, 2 * G], BF16)
            nc.sync.dma_start(b01_sb[:], bias01[:])
            pwown_sb = constp.tile([128, 4], BF16)
            nc.sync.dma_start(pwown_sb[:], pwown[:])
            msb2_sb = constp.tile([128, 2], F32)
            nc.sync.dma_start(msb2_sb[:], msb2[:])
            ones_sb = constp.tile([1, B], BF16)
            nc.any.memset(ones_sb[:], 1.0)
            ident = constp.tile([128, 128], BF16)
            make_identity(nc, ident[:])
            c0 = constp.tile([128, HS], F32)
            nc.vector.memset(c0[:], 0.0)
            c1 = constp.tile([128, HS], F32)
            nc.vector.memset(c1[:], 0.0)
            outbuf = constp.tile([128, 2 * T_steps], F32)

            def wchunk(c):
                return wall_sb[:, c * G:(c + 1) * G]

            def prefetch(t):
                fb = streamp.tile([128, KF * B], BF16, tag="feat", name="fb")
                nc.sync.dma_start(
                    fb[:].rearrange("p (k b) -> p k b", k=KF),
                    featT[t].rearrange("(k p) b -> p k b", p=128),
                )
                lb = streamp.tile([E, B], BF16, tag="lab", name="lb")
                nc.sync.dma_start(lb[:], labpreT[t])
                sc = streamp.tile([128, 1], BF16, tag="scol", name="sc")
                nc.sync.dma_start(sc[:], sT16[t:t + 1, :].rearrange("a b -> b a"))
                return fb, lb, sc

            def lstm_nonlin(gp, c):
                sg = workp.tile([128, 3 * HS], F32, tag="sg", name="sg")
                nc.scalar.activation(sg[:], gp[:, 0:3 * HS], AF.Sigmoid)
                tg = workp.tile([128, HS], F32, tag="tg", name="tg")
                nc.scalar.activation(tg[:], gp[:, 3 * HS:4 * HS], AF.Tanh)
                t1 = workp.tile([128, HS], F32, tag="t1", name="t1")
                nc.vector.tensor_mul(t1[:], sg[:, HS:2 * HS], c[:])
                t2 = workp.tile([128, HS], F32, tag="t2", name="t2")
                nc.vector.tensor_mul(t2[:], sg[:, 0:HS], tg[:])
                nc.vector.tensor_add(c[:], t1[:], t2[:])
                tc_ = workp.tile([128, HS], F32, tag="tc", name="tc_")
                nc.scalar.activation(tc_[:], c[:], AF.Tanh)
                hb = workp.tile([128, HS], BF16, tag="hb", name="hb")
                nc.vector.tensor_mul(hb[:], sg[:, 2 * HS:3 * HS], tc_[:])
                return hb

            def ag(agin, tagbase, width):
                agout = dramp.tile([H, width], BF16, tag=tagbase + "out",
                                   addr_space="Shared", name="agout")
                if mode == "full":
                    nc.gpsimd.collective_compute(
                        "AllGather",
                        mybir.AluOpType.bypass,
                        replica_groups=RG,
                        ins=[agin.opt()],
                        outs=[agout.opt()],
                    )
                elif mode == "noag":
                    nc.sync.dma_start(agout[0:128, :], agin[:])
                return agout

            # ---- preamble: pre-accumulate gates0 for t=0 ----
            fb_cur, lb_cur, sc_cur = prefetch(0)
            g0p = psump.tile([128, G], F32, tag="g0", name="g0p")
            nc.tensor.matmul(g0p[:], ones_sb[:], b01_sb[:, 0:G], start=True, stop=False)
            for k in range(KF):
                nc.tensor.matmul(g0p[:], fb_cur[:, ts(k, B)], wchunk(C_IH0 + k),
                                 start=False, stop=False)
            nc.tensor.matmul(g0p[:], lb_cur[:], wlab_sb[:], start=False, stop=True)

            h1T_prev = None
            sum_prev = None

            for t in range(T_steps):
                last = t == T_steps - 1
                if t > 0:
                    # mu feedback as DVE rank-1 updates:
                    # g0 += ubc*(s*mu_raw) + vbc*s   (mu_b folded into vbc)
                    smu = workp.tile([128, 1], F32, tag="smu", name="smu")
                    nc.vector.tensor_mul(smu[:], sc_cur[:], sum_prev[:, 0:1])
                    nc.vector.scalar_tensor_tensor(
                        g0p[:], ubc_sb[:], smu[:], g0p[:], ALU.mult, ALU.add)
                    nc.vector.scalar_tensor_tensor(
                        g0p[:], vbc_sb[:], sc_cur[:], g0p[:], ALU.mult, ALU.add)

                hb0 = lstm_nonlin(g0p, c0)
                trp0 = psump.tile([128, 128], BF16, tag="tr", name="trp0")
                nc.tensor.transpose(trp0[:], hb0[:], ident[:])
                hTs0 = workp.tile([128, 128], BF16, tag="hT", name="hTs0")
                nc.vector.tensor_copy(hTs0[:], trp0[:])
                agin0 = dramp.tile([128, B], BF16, tag="x0in", name="agin0")
                nc.sync.dma_start(agin0[:], hTs0[:])
                ag0out = ag(agin0, "x0", B)

                # hidden work while AG0 is in flight
                g1p = psump.tile([128, G], F32, tag="g1", name="g1p")
                nc.tensor.matmul(g1p[:], ones_sb[:], b01_sb[:, G:2 * G],
                                 start=True, stop=False)
                if h1T_prev is not None:
                    for k in range(KH):
                        nc.tensor.matmul(g1p[:], h1T_prev[:, ts(k, 128)],
                                         wchunk(C_HH1 + k), start=False, stop=False)

                h0T = statep.tile([128, H], BF16, tag="h0T", name="hTh0T")
                # land in two halves so the first Wih1 matmuls overlap the
                # second half of the 256KB landing DMA
                ag0v = ag0out.rearrange("(k p) b -> p k b", p=128)
                h0Tv = h0T[:].rearrange("p (k b) -> p k b", k=KH)
                for q in range(4):
                    nc.sync.dma_start(h0Tv[:, 2 * q:2 * q + 2, :],
                                      ag0v[:, 2 * q:2 * q + 2, :])
                for k in range(KH):
                    nc.tensor.matmul(g1p[:], h0T[:, ts(k, 128)], wchunk(C_IH1 + k),
                                     start=False, stop=(k == KH - 1))

                hb1 = lstm_nonlin(g1p, c1)
                # h1 message: [h1_j^T | own-slice mu/sig partial]
                trp1 = psump.tile([128, 128], BF16, tag="tr", name="trp1")
                nc.tensor.transpose(trp1[:], hb1[:], ident[:])
                msg = workp.tile([128, MW], BF16, tag="msg", name="msg")
                nc.vector.tensor_copy(msg[:, 0:B], trp1[:])
                part = psump.tile([128, 2], F32, tag="ms", name="part")
                nc.tensor.matmul(part[:], hTs0[:], pwown_sb[:, 0:2],
                                 start=True, stop=False)
                nc.tensor.matmul(part[:], msg[:, 0:B], pwown_sb[:, 2:4],
                                 start=False, stop=True)
                nc.vector.tensor_copy(msg[:, B:MW], part[:])
                agin1 = dramp.tile([128, MW], BF16, tag="x1in", name="agin1")
                nc.sync.dma_start(agin1[:], msg[:])
                ag1out = ag(agin1, "x1", MW)

                # hidden work while AG1 is in flight
                if not last:
                    fb_nxt, lb_nxt, sc_nxt = prefetch(t + 1)
                    g0p_nxt = psump.tile([128, G], F32, tag="g0", name="g0p")
                    nc.tensor.matmul(g0p_nxt[:], ones_sb[:], b01_sb[:, 0:G],
                                     start=True, stop=False)
                    for k in range(KF):
                        nc.tensor.matmul(g0p_nxt[:], fb_nxt[:, ts(k, B)],
                                         wchunk(C_IH0 + k), start=False, stop=False)
                    nc.tensor.matmul(g0p_nxt[:], lb_nxt[:], wlab_sb[:],
                                     start=False, stop=False)
                    for k in range(KH):
                        nc.tensor.matmul(g0p_nxt[:], h0T[:, ts(k, 128)],
                                         wchunk(C_HH0 + k), start=False,
                                         stop=(k == KH - 1))

                # mu/sigma: gather the 8 appended partials (4KB) and reduce —
                # the 256KB h1 landing below is NOT on this path
                psb = workp.tile([128, 2 * NCORE], BF16, tag="psb", name="psb")
                nc.sync.dma_start(
                    psb[:].rearrange("p (j c) -> p j c", c=2),
                    ag1out.rearrange("(j p) c -> p j c", p=128)[:, :, B:MW],
                )
                summed = workp.tile([128, 2], F32, tag="summed", name="summed")
                nc.vector.tensor_reduce(
                    summed[:],
                    psb[:].rearrange("p (j c) -> p c j", c=2),
                    mybir.AxisListType.X,
                    ALU.add,
                )
                nc.vector.tensor_add(outbuf[:, 2 * t:2 * t + 2], summed[:], msb2_sb[:])

                h1T = statep.tile([128, H], BF16, tag="h1T", name="hTh1T")
                nc.sync.dma_start(
                    h1T[:].rearrange("p (k b) -> p k b", k=KH),
                    ag1out[:, 0:B].rearrange("(k p) b -> p k b", p=128),
                )

                h1T_prev = h1T
                sum_prev = summed
                if not last:
                    fb_cur, lb_cur, sc_cur = fb_nxt, lb_nxt, sc_nxt
                    g0p = g0p_nxt

            nc.sync.dma_start(out_ms[:], outbuf[:])

    nc.compile()
    return nc


def make_in_maps(inputs, T_steps=T):
    """Host-side preprocessing: full numpy inputs -> per-core input dicts."""
    feat = np.asarray(inputs["feat"], np.float32)
    lab = np.asarray(inputs["embedded_labels"], np.float32)
    mask = np.asarray(inputs["mask"])
    W_ih0 = np.asarray(inputs["W_ih0"], np.float32)
    W_hh0 = np.asarray(inputs["W_hh0"], np.float32)
    b_ih0 = np.asarray(inputs["b_ih0"], np.float32)
    b_hh0 = np.asarray(inputs["b_hh0"], np.float32)
    W_ih1 = np.asarray(inputs["W_ih1"], np.float32)
    W_hh1 = np.asarray(inputs["W_hh1"], np.float32)
    b_ih1 = np.asarray(inputs["b_ih1"], np.float32)
    b_hh1 = np.asarray(inputs["b_hh1"], np.float32)
    W_tgt = np.asarray(inputs["W_tgt"], np.float32)
    b_tgt = np.asarray(inputs["b_tgt"], np.float32)
    mu_w = np.asarray(inputs["mu_w"], np.float32)
    mu_b = np.asarray(inputs["mu_b"], np.float32)
    sig_w = np.asarray(inputs["sig_w"], np.float32)
    sig_b = np.asarray(inputs["sig_b"], np.float32)

    s = mask.T.astype(np.float32).copy()  # [T, B]
    s[0, :] = 0.0
    featT = np.ascontiguousarray(feat.transpose(1, 2, 0))[:T_steps]      # [T, F, B]
    labT = lab.transpose(1, 2, 0)                                        # [T, E, B]
    labpreT = np.ascontiguousarray(labT * (1.0 - s)[:, None, :])[:T_steps]
    sT = np.ascontiguousarray(s[:T_steps])

    mu_e, mu_o = mu_w[0::2, 0], mu_w[1::2, 0]
    sig_e, sig_o = sig_w[0::2, 0], sig_w[1::2, 0]
    msb2 = np.tile(np.array([[mu_b[0], sig_b[0]]], np.float32), (128, 1))

    bf = ml_dtypes.bfloat16
    in_maps = []
    for j in range(NCORE):
        r0 = gate_rows(j)
        r1 = gate_rows(j)
        chunks = []
        for k in range(KF):
            chunks.append(W_ih0[r0][:, 128 * k:128 * (k + 1)].T)
        for k in range(KH):
            chunks.append(W_hh0[r0][:, 128 * k:128 * (k + 1)].T)
        for k in range(KH):
            chunks.append(W_ih1[r1][:, 128 * k:128 * (k + 1)].T)
        for k in range(KH):
            chunks.append(W_hh1[r1][:, 128 * k:128 * (k + 1)].T)
        wall = np.ascontiguousarray(np.concatenate(chunks, axis=1), dtype=np.float32)
        wlab_j = np.ascontiguousarray(W_ih0[r0][:, F:].T)           # [E, G]
        u_j = W_ih0[r0][:, F:] @ W_tgt[0]                           # [G]
        v_j = W_ih0[r0][:, F:] @ b_tgt + float(mu_b[0]) * u_j       # mu_b folded
        ubc_j = np.tile(u_j[None, :], (128, 1))
        vbc_j = np.tile(v_j[None, :], (128, 1))
        sl = slice(HS * j, HS * j + HS)
        pwown_j = np.stack([mu_e[sl], sig_e[sl], mu_o[sl], sig_o[sl]], axis=1)
        bias01_j = np.concatenate([(b_ih0 + b_hh0)[r0], (b_ih1 + b_hh1)[r1]])[None, :]
        in_maps.append({
            "wall": wall.astype(bf),
            "wlab": wlab_j.astype(bf),
            "ubc": ubc_j.astype(bf),
            "vbc": vbc_j.astype(bf),
            "bias01": np.ascontiguousarray(bias01_j, dtype=np.float32).astype(bf),
            "pwown": np.ascontiguousarray(pwown_j, dtype=np.float32).astype(bf),
            "msb2": msb2,
            "featT": featT.astype(bf),
            "labpreT": labpreT.astype(bf),
            "sT16": sT.astype(bf),
        })
    return in_maps


def postprocess(out_ms, T_steps=T):
    out = out_ms.reshape(128, T_steps, 2)
    mu = out[:, :, 0]
    sig = np.logaddexp(0.0, out[:, :, 1])
    return np.stack([mu, sig], axis=-1).astype(np.float32)


_BUILT = {}


def kernel(**inputs):
    in_maps = make_in_maps(inputs)
    if T not in _BUILT:
        _BUILT[T] = build(T)
    nc = _BUILT[T]
    res = run_bass_kernel_spmd(nc, in_maps, list(range(NCORE)))
    return postprocess(res.results[0]["out_ms"])
